# revision 25
# baseline (speedup 1.0000x reference)
"""CRF negative-log-likelihood loss kernel for Trainium2 (8 NeuronCores).

Strategy
--------
Data-parallel over the batch: 32 sequences -> 4 per core. Each core evaluates
the log-partition function in *linear* space with a bidirectional split that
halves the serial chain: the forward recurrence

    alpha_t = (M @ alpha_{t-1}) * e_t,      M = exp(T), e_t = exp(feat_t)

runs from t=0 up to t=255 while the backward recurrence

    beta_t  = M^T-contraction of (e_{t+1} * beta_{t+1}),   beta_511 = 1

runs from t=511 down to t=255; both are one 64x64 TensorEngine matmul plus
one elementwise VectorEngine multiply per step, and the two chains interleave
on the engines so the wall time is one chain's ~256-step latency. They meet
with  Z = sum_i alpha_255[i] * beta_255[i].

The matmul weights are augmented to [W | ONES] (bf16, single PE pass), so
every step's psum also delivers sum_i(state) broadcast across rows 64-127.
Every K=8 steps that sum renormalizes the chain: the reciprocal is folded
into a later step's emission operand (scale-invariance), so normalization
never touches the serial critical path. Each log-scale is evaluated as
Ln(s * 2^-48) (ScalarE Ln saturates at 2^64) and the 48*ln2 is added back at
the end.

The gold-path score (emissions at tags plus transitions) is computed with
one-hot matmuls: per sequence, G = [F | OH_next]^T @ OH_prev has feats^T@OH
in rows 0-63 (diagonal = emission score) and the transition-pair count matrix
in rows 64-127 (Frobenius product with T = transition score); one multiply
with [I; T], a row reduce, and a ones-matmul collapse it to scalars. All off
the critical path.

Host-side work is limited to input relayout: per-core slicing, one-hot
encoding of the integer tags (with a zero guard row), transposing T, and
concatenating eye(64) with T.
"""

import math

import numpy as np
from contextlib import ExitStack

B, T_LEN, L = 32, 512, 64
N_CORES = 8
BPC = B // N_CORES  # sequences per core
T_MID = 255         # chains meet here
K_NORM = 8          # renormalize every K steps
LN_SCALE = 2.0 ** -48

_compiled = None  # compiled program cache so repeated kernel() calls reuse it


def _build_program():
    import concourse.bacc as bacc
    import concourse.tile as tile
    import concourse.mybir as mybir
    from concourse.alu_op_type import AluOpType

    f32 = mybir.dt.float32
    bf16 = mybir.dt.bfloat16
    Af = mybir.ActivationFunctionType

    nc = bacc.Bacc("TRN2", target_bir_lowering=False, debug=False,
                   num_devices=N_CORES)

    feats_d = nc.dram_tensor("feats", [BPC * T_LEN, L], f32,
                             kind="ExternalInput").ap()
    oh_d = nc.dram_tensor("oh", [BPC * (T_LEN + 1), L], f32,
                          kind="ExternalInput").ap()
    tt_d = nc.dram_tensor("tt", [L, L], f32, kind="ExternalInput").ap()
    mask_d = nc.dram_tensor("mask", [2 * L, L], f32, kind="ExternalInput").ap()
    out_d = nc.dram_tensor("out", [1, BPC], f32, kind="ExternalOutput").ap()

    with tile.TileContext(nc) as tc, ExitStack() as ctx:
        consts = ctx.enter_context(tc.tile_pool(name="consts", bufs=1))
        loadp = ctx.enter_context(tc.tile_pool(name="load", bufs=4))
        alphap = ctx.enter_context(tc.tile_pool(name="alpha", bufs=4))
        vtmp = ctx.enter_context(tc.tile_pool(name="vtmp", bufs=6))
        qf = ctx.enter_context(tc.tile_pool(name="qfpsum", bufs=3, space="PSUM"))
        qb = ctx.enter_context(tc.tile_pool(name="qbpsum", bufs=3, space="PSUM"))
        tpp = ctx.enter_context(tc.tile_pool(name="tpsum", bufs=2, space="PSUM"))

        # ---- constants ----
        ones128 = consts.tile([128, 1], f32)
        nc.gpsimd.memset(ones128[:], 1.0)
        mask_sb = consts.tile([128, L], f32)
        nc.sync.dma_start(out=mask_sb[:], in_=mask_d)
        ttile = consts.tile([L, L], f32)          # T^T
        nc.sync.dma_start(out=ttile[:], in_=tt_d)
        tstr = consts.tile([L, L], f32)           # T (straight)
        nc.sync.dma_start(out=tstr[:], in_=mask_d[L:2 * L, :])
        # W3  = [exp(T)^T | ONES]  (forward);  W3b = [exp(T) | ONES] (backward)
        # Matmul against either gives the new state in psum rows 0-63 and the
        # input-state column sums broadcast across rows 64-127.
        W3 = consts.tile([L, 2 * L], bf16)
        nc.scalar.activation(W3[:, 0:L], ttile[:], Af.Exp)
        nc.gpsimd.memset(W3[:, L:2 * L], 1.0)
        W3b = consts.tile([L, 2 * L], bf16)
        nc.scalar.activation(W3b[:, 0:L], tstr[:], Af.Exp)
        nc.gpsimd.memset(W3b[:, L:2 * L], 1.0)

        # ---- e_feats in t-major chunks: efc[k][j, (t%32)*4 + b] (bf16) ----
        # Per chunk: contiguous-ish DMA of 128 t-major rows, Exp -> bf16 into
        # the left half of a [128,128] staging tile, then an xbar
        # DMA-transpose (2-byte dtype, free%128) whose partitions 0-63 are the
        # transposed chunk. No TensorEngine involvement, so the recurrence
        # matmuls never hit a PE tiling-mode switch. Chunks are emitted in the
        # order the two chains consume them (0, 15, 1, 14, ...).
        feats_tmaj = feats_d.rearrange("(b t) l -> t b l", b=BPC)  # [512,4,64]
        efc = [None] * 16
        order = []
        for k in range(8):
            order += [k, 15 - k]
        for k in order:
            fc = loadp.tile([128, L], f32, tag="fchunk")
            nc.sync.dma_start(out=fc[:],
                              in_=feats_tmaj[k * 32:(k + 1) * 32, :, :])
            stg = loadp.tile([128, 128], bf16, tag="stg")
            nc.gpsimd.memset(stg[:, L:128], 0.0)
            nc.scalar.activation(stg[:, 0:L], fc[:], Af.Exp)
            ek = consts.tile([128, 128], bf16, tag=f"ef{k}")
            nc.sync.dma_start(out=ek[:], in_=stg[:], transpose=True)
            efc[k] = ek

        def ef_col(t):  # [64, 4] AP of exp(feats[:, t, :]) for the 4 seqs
            return efc[t // 32][0:L, 4 * (t % 32):4 * (t % 32) + 4]

        # ---- bidirectional recurrence ----
        n_events = 65  # 31+1 fwd, 31+1 bwd, 1 final combine
        lnS = consts.tile([1, 4 * n_events], f32)
        ev = 0

        def emit_ln(ps_row):  # ps_row: [1, BPC] psum AP holding s
            nonlocal ev
            nc.scalar.activation(lnS[:, 4 * ev:4 * ev + 4], ps_row,
                                 Af.Ln, scale=LN_SCALE)
            ev += 1

        alpha = alphap.tile([L, BPC], bf16, tag="alpha")
        nc.vector.tensor_copy(alpha[:], ef_col(0))
        v = alphap.tile([L, BPC], bf16, tag="v")
        nc.vector.tensor_copy(v[:], ef_col(T_LEN - 1))

        es_f = {}   # fwd step -> prescaled emission operand
        es_b = {}   # bwd step -> prescaled emission operand
        fwd_events = set(range(K_NORM, T_MID - K_NORM + 2, K_NORM)) | {T_MID}
        bwd_events = (set(range(T_MID + K_NORM + 1, T_LEN - K_NORM + 1,
                                K_NORM)) | {T_MID + 1})

        for s in range(T_MID):
            tf = 1 + s          # forward step index
            tb = T_LEN - 2 - s  # backward step index (mul at tb)

            # forward: q = W3^T @ alpha ; alpha = q[0:64] * e
            q = qf.tile([2 * L, BPC], f32, tag="q")
            nc.tensor.matmul(q[:], lhsT=W3[:], rhs=alpha[:],
                             start=True, stop=True)
            eop = es_f.pop(tf, None)
            if eop is None:
                eop = ef_col(tf)
            alpha_new = alphap.tile([L, BPC], bf16, tag="alpha")
            nc.vector.tensor_mul(alpha_new[:], q[0:L, :], eop)
            alpha = alpha_new
            if tf + 1 in fwd_events:  # 1/s(alpha_{tf-1}) lands at step tf+1
                rvf = vtmp.tile([L, BPC], f32, tag="rvf")
                nc.vector.reciprocal(rvf[:], q[L:2 * L, :])
                esf = vtmp.tile([L, BPC], f32, tag="esf")
                nc.gpsimd.tensor_mul(esf[:], ef_col(tf + 1), rvf[:])
                emit_ln(q[L:L + 1, :])
                es_f[tf + 1] = esf

            # backward: p = W3b^T @ v_{tb+1} ; v_tb = p[0:64] * e_tb
            p = qb.tile([2 * L, BPC], f32, tag="p")
            nc.tensor.matmul(p[:], lhsT=W3b[:], rhs=v[:],
                             start=True, stop=True)
            eop = es_b.pop(tb, None)
            if eop is None:
                eop = ef_col(tb)
            v_new = alphap.tile([L, BPC], bf16, tag="v")
            nc.vector.tensor_mul(v_new[:], p[0:L, :], eop)
            v = v_new
            if tb - 1 in bwd_events:
                rvb = vtmp.tile([L, BPC], f32, tag="rvb")
                nc.vector.reciprocal(rvb[:], p[L:2 * L, :])
                esb = vtmp.tile([L, BPC], f32, tag="esb")
                nc.gpsimd.tensor_mul(esb[:], ef_col(tb - 1), rvb[:])
                emit_ln(p[L:L + 1, :])
                es_b[tb - 1] = esb

        assert not es_f and not es_b, (sorted(es_f), sorted(es_b))
        # last backward contraction down to T_MID (no emission at T_MID here:
        # alpha_255 already carries e_255)
        p = qb.tile([2 * L, BPC], f32, tag="p")
        nc.tensor.matmul(p[:], lhsT=W3b[:], rhs=v[:], start=True, stop=True)

        # combine: Z_core = sum_i alpha_255[i] * beta_255[i]
        g = alphap.tile([L, BPC], bf16, tag="alpha")
        nc.vector.tensor_mul(g[:], p[0:L, :], alpha[:])
        qz = qf.tile([2 * L, BPC], f32, tag="q")
        nc.tensor.matmul(qz[:], lhsT=W3[:], rhs=g[:], start=True, stop=True)
        emit_ln(qz[L:L + 1, :])
        assert ev == n_events, ev

        fwd = vtmp.tile([1, BPC], f32, tag="fwd")
        nc.vector.tensor_reduce(
            fwd[:], lnS[:].rearrange("p (n b) -> p b n", b=BPC),
            axis=mybir.AxisListType.X, op=AluOpType.add)
        # add back the n_events * 48*ln2 removed by the Ln pre-scale
        lnoff = consts.tile([1, BPC], f32)
        nc.gpsimd.memset(lnoff[:], float(n_events * 48.0 * math.log(2.0)))
        fwd2 = vtmp.tile([1, BPC], f32, tag="fwd2")
        nc.vector.tensor_add(fwd2[:], fwd[:], lnoff[:])

        # ---- gold score via one-hot matmuls ----
        # Each [128,128]^T@[128,64] product is split into two 64-row halves on
        # PE quadrants T0/T8 so these stay in the loop's (64,128) tiling mode
        # (no PE drain when they fill loop idle slots).
        # The [128,.]^T@[128,.] products are emitted as two 64-contraction
        # halves packed side by side in partitions 0-63 (cat2 cols
        # [Flo|OHNlo|Fhi|OHNhi], ohp2 cols [OHPlo|OHPhi]) so every gold
        # matmul uses the same (64,128) PE tiling mode as the recurrence and
        # can fill its idle slots without a mode-switch drain.
        Vt = consts.tile([128, BPC], f32)
        for b in range(BPC):
            gps = tpp.tile([128, L], f32, tag="tp")
            for c in range(4):
                r0 = b * T_LEN + c * 128
                o0 = b * (T_LEN + 1) + c * 128
                cat = loadp.tile([128, 128], f32, tag="cat")
                nc.sync.dma_start(out=cat[:, 0:L],
                                  in_=feats_d[r0:r0 + 128, :])
                nc.sync.dma_start(out=cat[:, L:2 * L],
                                  in_=oh_d[o0 + 1:o0 + 129, :])
                ohp = loadp.tile([128, L], f32, tag="ohp")
                nc.sync.dma_start(out=ohp[:], in_=oh_d[o0:o0 + 128, :])
                nc.tensor.matmul(gps[:], lhsT=cat[:], rhs=ohp[:],
                                 start=(c == 0), stop=(c == 3))
            gsc = vtmp.tile([128, L], f32, tag="gsc")
            nc.vector.tensor_mul(gsc[:], gps[:], mask_sb[:])
            nc.vector.tensor_reduce(Vt[:, b:b + 1], gsc[:],
                                    axis=mybir.AxisListType.X,
                                    op=AluOpType.add)
        gold_ps = tpp.tile([128, L], f32, tag="tp")
        nc.tensor.matmul(gold_ps[0:1, 0:BPC], lhsT=ones128[:, 0:1], rhs=Vt[:],
                         start=True, stop=True)

        res = vtmp.tile([1, BPC], f32, tag="res")
        nc.vector.tensor_tensor(res[:], fwd2[:], gold_ps[0:1, 0:BPC],
                                op=AluOpType.subtract)
        nc.sync.dma_start(out=out_d, in_=res[:])

    nc.compile()
    return nc


def _prep_in_maps(feats, tags, T):
    feats = np.ascontiguousarray(np.asarray(feats, dtype=np.float32))
    T_np = np.ascontiguousarray(np.asarray(T, dtype=np.float32))
    tags_np = np.asarray(tags).astype(np.int64)

    oh = np.zeros((B, T_LEN + 1, L), dtype=np.float32)
    oh[np.arange(B)[:, None], np.arange(T_LEN)[None, :], tags_np] = 1.0
    mask_const = np.concatenate([np.eye(L, dtype=np.float32), T_np], axis=0)
    tt = np.ascontiguousarray(T_np.T)

    in_maps = []
    for c in range(N_CORES):
        sl = slice(c * BPC, (c + 1) * BPC)
        in_maps.append({
            "feats": np.ascontiguousarray(
                feats[sl].reshape(BPC * T_LEN, L)),
            "oh": np.ascontiguousarray(
                oh[sl].reshape(BPC * (T_LEN + 1), L)),
            "tt": tt,
            "mask": mask_const,
        })
    return in_maps


def kernel(feats, tags, T):
    global _compiled
    from concourse.bass_utils import run_bass_kernel_spmd

    if _compiled is None:
        _compiled = _build_program()
    nc = _compiled

    in_maps = _prep_in_maps(feats, tags, T)
    res = run_bass_kernel_spmd(nc, in_maps, list(range(N_CORES)))
    out = np.concatenate(
        [res.results[c]["out"].reshape(BPC) for c in range(N_CORES)])
    return out.astype(np.float32)


# revision 26
# speedup vs baseline: 1.1546x; 1.1546x over previous
"""CRF negative-log-likelihood loss kernel for Trainium2 (8 NeuronCores).

Strategy
--------
Data-parallel over the batch: 32 sequences -> 4 per core. Each core evaluates
the log-partition function in *linear* space with a bidirectional split that
halves the serial chain: the forward recurrence

    alpha_t = (M @ alpha_{t-1}) * e_t,      M = exp(T), e_t = exp(feat_t)

runs from t=0 up to t=255 while the backward recurrence

    beta_t  = M^T-contraction of (e_{t+1} * beta_{t+1}),   beta_511 = 1

runs from t=511 down to t=255; both are one 64x64 TensorEngine matmul plus
one elementwise VectorEngine multiply per step, and the two chains interleave
on the engines so the wall time is one chain's ~256-step latency. They meet
with  Z = sum_i alpha_255[i] * beta_255[i].

The matmul weights are augmented to [W | ONES] (bf16, single PE pass), so
every step's psum also delivers sum_i(state) broadcast across rows 64-127.
Every K=8 steps that sum renormalizes the chain: the reciprocal is folded
into a later step's emission operand (scale-invariance), so normalization
never touches the serial critical path. Each log-scale is evaluated as
Ln(s * 2^-48) (ScalarE Ln saturates at 2^64) and the 48*ln2 is added back at
the end.

The gold-path score (emissions at tags plus transitions) is computed with
one-hot matmuls: per sequence, G = [F | OH_next]^T @ OH_prev has feats^T@OH
in rows 0-63 (diagonal = emission score) and the transition-pair count matrix
in rows 64-127 (Frobenius product with T = transition score); one multiply
with [I; T], a row reduce, and a ones-matmul collapse it to scalars. All off
the critical path.

Host-side work is limited to input relayout: per-core slicing, one-hot
encoding of the integer tags (with a zero guard row), transposing T, and
concatenating eye(64) with T.
"""

import math

import numpy as np
from contextlib import ExitStack

B, T_LEN, L = 32, 512, 64
N_CORES = 8
BPC = B // N_CORES  # sequences per core
T_MID = 255         # chains meet here
K_NORM = 8          # renormalize every K steps
LN_SCALE = 2.0 ** -48

_compiled = None  # compiled program cache so repeated kernel() calls reuse it


def _build_program():
    import concourse.bacc as bacc
    import concourse.tile as tile
    import concourse.mybir as mybir
    from concourse.alu_op_type import AluOpType

    f32 = mybir.dt.float32
    bf16 = mybir.dt.bfloat16
    Af = mybir.ActivationFunctionType

    nc = bacc.Bacc("TRN2", target_bir_lowering=False, debug=False,
                   num_devices=N_CORES)

    feats_d = nc.dram_tensor("feats", [BPC * T_LEN, L], f32,
                             kind="ExternalInput").ap()
    goffs_d = nc.dram_tensor("goffs", [128, 32], mybir.dt.int32,
                             kind="ExternalInput").ap()
    tt_d = nc.dram_tensor("tt", [L, L], f32, kind="ExternalInput").ap()
    mask_d = nc.dram_tensor("mask", [2 * L, L], f32, kind="ExternalInput").ap()
    out_d = nc.dram_tensor("out", [1, BPC], f32, kind="ExternalOutput").ap()

    with tile.TileContext(nc) as tc, ExitStack() as ctx:
        consts = ctx.enter_context(tc.tile_pool(name="consts", bufs=1))
        loadp = ctx.enter_context(tc.tile_pool(name="load", bufs=4))
        alphap = ctx.enter_context(tc.tile_pool(name="alpha", bufs=4))
        vtmp = ctx.enter_context(tc.tile_pool(name="vtmp", bufs=6))
        qf = ctx.enter_context(tc.tile_pool(name="qfpsum", bufs=3, space="PSUM"))
        qb = ctx.enter_context(tc.tile_pool(name="qbpsum", bufs=3, space="PSUM"))
        tpp = ctx.enter_context(tc.tile_pool(name="tpsum", bufs=2, space="PSUM"))

        # ---- constants ----
        from concourse import masks
        ident64 = consts.tile([L, L], f32)
        masks.make_identity(nc, ident64[:])
        ttile = consts.tile([L, L], f32)          # T^T
        nc.sync.dma_start(out=ttile[:], in_=tt_d)
        tstr = consts.tile([L, L], f32)           # T (straight)
        nc.sync.dma_start(out=tstr[:], in_=mask_d[L:2 * L, :])
        # W3  = [exp(T)^T | ONES]  (forward);  W3b = [exp(T) | ONES] (backward)
        # Matmul against either gives the new state in psum rows 0-63 and the
        # input-state column sums broadcast across rows 64-127.
        W3 = consts.tile([L, 2 * L], bf16)
        nc.scalar.activation(W3[:, 0:L], ttile[:], Af.Exp)
        nc.gpsimd.memset(W3[:, L:2 * L], 1.0)
        W3b = consts.tile([L, 2 * L], bf16)
        nc.scalar.activation(W3b[:, 0:L], tstr[:], Af.Exp)
        nc.gpsimd.memset(W3b[:, L:2 * L], 1.0)

        # ---- e_feats in t-major chunks: efc[k][j, (t%32)*4 + b] (bf16) ----
        # Per chunk: contiguous-ish DMA of 128 t-major rows, Exp -> bf16 into
        # the left half of a [128,128] staging tile, then an xbar
        # DMA-transpose (2-byte dtype, free%128) whose partitions 0-63 are the
        # transposed chunk. No TensorEngine involvement, so the recurrence
        # matmuls never hit a PE tiling-mode switch. Chunks are emitted in the
        # order the two chains consume them (0, 15, 1, 14, ...).
        feats_tmaj = feats_d.rearrange("(b t) l -> t b l", b=BPC)  # [512,4,64]
        efc = [None] * 16
        order = []
        for k in range(8):
            order += [k, 15 - k]
        for k in order:
            ek = consts.tile([L, 128], f32, tag=f"ef{k}")
            for h in range(2):
                fc = loadp.tile([L, 128], f32, tag="fchunk")
                nc.gpsimd.memset(fc[:, L:128], 0.0)
                r0 = k * 32 + 16 * h
                nc.sync.dma_start(out=fc[:, 0:L],
                                  in_=feats_tmaj[r0:r0 + 16, :, :])
                ps = tpp.tile([128, L], f32, tag="tp")
                nc.tensor.transpose(ps[:], fc[:], ident64[:])
                nc.scalar.activation(ek[:, 64 * h:64 * h + 64],
                                     ps[0:L, :], Af.Exp)
            efc[k] = ek

        def ef_col(t):  # [64, 4] AP of exp(feats[:, t, :]) for the 4 seqs
            return efc[t // 32][:, 4 * (t % 32):4 * (t % 32) + 4]

        # ---- bidirectional recurrence ----
        n_events = 65  # 31+1 fwd, 31+1 bwd, 1 final combine
        lnS = consts.tile([1, 4 * n_events], f32)
        ev = 0

        def emit_ln(ps_row):  # ps_row: [1, BPC] psum AP holding s
            nonlocal ev
            nc.scalar.activation(lnS[:, 4 * ev:4 * ev + 4], ps_row,
                                 Af.Ln, scale=LN_SCALE)
            ev += 1

        alpha = alphap.tile([L, BPC], bf16, tag="alpha")
        nc.vector.tensor_copy(alpha[:], ef_col(0))
        v = alphap.tile([L, BPC], bf16, tag="v")
        nc.vector.tensor_copy(v[:], ef_col(T_LEN - 1))

        es_f = {}   # fwd step -> prescaled emission operand
        es_b = {}   # bwd step -> prescaled emission operand
        fwd_events = set(range(K_NORM, T_MID - K_NORM + 2, K_NORM)) | {T_MID}
        bwd_events = (set(range(T_MID + K_NORM + 1, T_LEN - K_NORM + 1,
                                K_NORM)) | {T_MID + 1})

        for s in range(T_MID):
            tf = 1 + s          # forward step index
            tb = T_LEN - 2 - s  # backward step index (mul at tb)

            # forward: q = W3^T @ alpha ; alpha = q[0:64] * e
            q = qf.tile([2 * L, BPC], f32, tag="q")
            nc.tensor.matmul(q[:], lhsT=W3[:], rhs=alpha[:],
                             start=True, stop=True)
            eop = es_f.pop(tf, None)
            if eop is None:
                eop = ef_col(tf)
            alpha_new = alphap.tile([L, BPC], bf16, tag="alpha")
            nc.vector.tensor_mul(alpha_new[:], q[0:L, :], eop)
            alpha = alpha_new
            if tf + 1 in fwd_events:  # 1/s(alpha_{tf-1}) lands at step tf+1
                rvf = vtmp.tile([L, BPC], f32, tag="rvf")
                nc.vector.reciprocal(rvf[:], q[L:2 * L, :])
                esf = vtmp.tile([L, BPC], f32, tag="esf")
                nc.gpsimd.tensor_mul(esf[:], ef_col(tf + 1), rvf[:])
                emit_ln(q[L:L + 1, :])
                es_f[tf + 1] = esf

            # backward: p = W3b^T @ v_{tb+1} ; v_tb = p[0:64] * e_tb
            p = qb.tile([2 * L, BPC], f32, tag="p")
            nc.tensor.matmul(p[:], lhsT=W3b[:], rhs=v[:],
                             start=True, stop=True)
            eop = es_b.pop(tb, None)
            if eop is None:
                eop = ef_col(tb)
            v_new = alphap.tile([L, BPC], bf16, tag="v")
            nc.vector.tensor_mul(v_new[:], p[0:L, :], eop)
            v = v_new
            if tb - 1 in bwd_events:
                rvb = vtmp.tile([L, BPC], f32, tag="rvb")
                nc.vector.reciprocal(rvb[:], p[L:2 * L, :])
                esb = vtmp.tile([L, BPC], f32, tag="esb")
                nc.gpsimd.tensor_mul(esb[:], ef_col(tb - 1), rvb[:])
                emit_ln(p[L:L + 1, :])
                es_b[tb - 1] = esb

        assert not es_f and not es_b, (sorted(es_f), sorted(es_b))
        # last backward contraction down to T_MID (no emission at T_MID here:
        # alpha_255 already carries e_255)
        p = qb.tile([2 * L, BPC], f32, tag="p")
        nc.tensor.matmul(p[:], lhsT=W3b[:], rhs=v[:], start=True, stop=True)

        # combine: Z_core = sum_i alpha_255[i] * beta_255[i]
        g = alphap.tile([L, BPC], bf16, tag="alpha")
        nc.vector.tensor_mul(g[:], p[0:L, :], alpha[:])
        qz = qf.tile([2 * L, BPC], f32, tag="q")
        nc.tensor.matmul(qz[:], lhsT=W3[:], rhs=g[:], start=True, stop=True)
        emit_ln(qz[L:L + 1, :])
        assert ev == n_events, ev

        fwd = vtmp.tile([1, BPC], f32, tag="fwd")
        nc.vector.tensor_reduce(
            fwd[:], lnS[:].rearrange("p (n b) -> p b n", b=BPC),
            axis=mybir.AxisListType.X, op=AluOpType.add)
        # add back the n_events * 48*ln2 removed by the Ln pre-scale
        lnoff = consts.tile([1, BPC], f32)
        nc.gpsimd.memset(lnoff[:], float(n_events * 48.0 * math.log(2.0)))
        fwd2 = vtmp.tile([1, BPC], f32, tag="fwd2")
        nc.vector.tensor_add(fwd2[:], fwd[:], lnoff[:])

        # ---- gold score via indirect-DMA gathers ----
        # Host-computed element offsets (from the integer tags): partition p
        # holds sequence b=p//32; emit cols 0:16 gather feats[b,t,tag[b,t]],
        # trans cols 16:32 gather T[tag[b,t+1],tag[b,t]] from tt (=T^T), with
        # out-of-bounds offsets silently skipped for the 4 pad slots.
        import concourse.bass as bass
        offs = consts.tile([128, 32], mybir.dt.int32)
        nc.sync.dma_start(out=offs[:], in_=goffs_d)
        gath = consts.tile([128, 32], f32)
        nc.gpsimd.memset(gath[:], 0.0)
        feats_flat = feats_d.rearrange("a (b one) -> (a b) one", one=1)
        tt_flat = tt_d.rearrange("a (b one) -> (a b) one", one=1)
        for c in range(16):
            nc.gpsimd.indirect_dma_start(
                out=gath[:, c:c + 1], out_offset=None,
                in_=feats_flat,
                in_offset=bass.IndirectOffsetOnAxis(ap=offs[:, c:c + 1],
                                                    axis=0))
            nc.gpsimd.indirect_dma_start(
                out=gath[:, 16 + c:17 + c], out_offset=None,
                in_=tt_flat,
                in_offset=bass.IndirectOffsetOnAxis(ap=offs[:, 16 + c:17 + c],
                                                    axis=0),
                bounds_check=L * L - 1, oob_is_err=False)
        grow = consts.tile([128, 1], f32)
        nc.vector.tensor_reduce(grow[:], gath[:], axis=mybir.AxisListType.X,
                                op=AluOpType.add)
        G4 = consts.tile([128, BPC], f32)
        nc.gpsimd.memset(G4[:], 0.0)
        for b in range(BPC):
            nc.gpsimd.memset(G4[32 * b:32 * (b + 1), b:b + 1], 1.0)
        gold_ps = tpp.tile([128, L], f32, tag="tp")
        nc.tensor.matmul(gold_ps[0:1, 0:BPC], lhsT=grow[:], rhs=G4[:],
                         start=True, stop=True)

        res = vtmp.tile([1, BPC], f32, tag="res")
        nc.vector.tensor_tensor(res[:], fwd2[:], gold_ps[0:1, 0:BPC],
                                op=AluOpType.subtract)
        nc.sync.dma_start(out=out_d, in_=res[:])

    nc.compile()
    return nc


def _prep_in_maps(feats, tags, T):
    feats = np.ascontiguousarray(np.asarray(feats, dtype=np.float32))
    T_np = np.ascontiguousarray(np.asarray(T, dtype=np.float32))
    tags_np = np.asarray(tags).astype(np.int64)

    mask_const = np.concatenate([np.eye(L, dtype=np.float32), T_np], axis=0)
    tt = np.ascontiguousarray(T_np.T)

    # gather offsets: partition p -> (b=p//32, s=p%32); col c -> t = c*32+s.
    # emit: flat element index into this core's feats [BPC*T_LEN*L];
    # trans: flat index into tt (= T^T), i.e. tag[b,t]*L + tag[b,t+1];
    # the four t=511 trans slots get an out-of-bounds sentinel (skipped).
    p_arr = np.arange(128)
    c_arr = np.arange(16)
    b_pc = (p_arr // 32)[:, None]
    t_pc = c_arr[None, :] * 32 + (p_arr % 32)[:, None]

    in_maps = []
    for c in range(N_CORES):
        sl = slice(c * BPC, (c + 1) * BPC)
        tg = tags_np[sl]
        emit_offs = ((b_pc * T_LEN + t_pc) * L
                     + tg[b_pc, t_pc]).astype(np.int32)
        tnext = np.minimum(t_pc + 1, T_LEN - 1)
        trans_offs = (tg[b_pc, t_pc] * L + tg[b_pc, tnext]).astype(np.int32)
        trans_offs[t_pc == T_LEN - 1] = 2 ** 20  # OOB sentinel -> skipped
        in_maps.append({
            "feats": np.ascontiguousarray(
                feats[sl].reshape(BPC * T_LEN, L)),
            "goffs": np.concatenate([emit_offs, trans_offs], axis=1),
            "tt": tt,
            "mask": mask_const,
        })
    return in_maps


def kernel(feats, tags, T):
    global _compiled
    from concourse.bass_utils import run_bass_kernel_spmd

    if _compiled is None:
        _compiled = _build_program()
    nc = _compiled

    in_maps = _prep_in_maps(feats, tags, T)
    res = run_bass_kernel_spmd(nc, in_maps, list(range(N_CORES)))
    out = np.concatenate(
        [res.results[c]["out"].reshape(BPC) for c in range(N_CORES)])
    return out.astype(np.float32)
